# revision 1
# baseline (speedup 1.0000x reference)
"""Trainium2 Bass kernel for nn_LGnet (LSTM + memory attention recurrence).

Sharding: data-parallel over batch, B=256 -> 32 rows per core across 8 cores.
All on-chip state is kept transposed ([feature partitions, batch free]) so the
recurrence never needs a transpose. The z/zp gating streams (input-only) and
their contribution to the attention query `ls` are precomputed in T-chunks
before the sequential loop; the loop itself runs 100 steps of:
  ls = ls_z[t] + h @ WQ3F.T        (4 fp32 matmuls)
  logits = memory @ ls             (4 fp32 matmuls)
  e = exp(logits); s = sum(e); gd = (e @ memory) / s   (bf16 matmuls)
  gates = Wcat.T @ [gd; h]         (80 bf16 matmuls, weights stationary)
  LSTM pointwise via tanh (sigmoid = 0.5*tanh(0.5x)+0.5, ACT exp-table set)
"""
import os
import numpy as np
from contextlib import ExitStack

B, T, F, H, O, M = 256, 100, 128, 512, 128, 512
T = int(os.environ.get("LG_T", str(T)))   # debug override; harness uses 100
NC = 8
BB = B // NC          # 32 batch rows per core
TB = T * BB           # 3200 columns in (t, b) packing
NTCH = 4              # precompute T-chunks
TCH = T // NTCH       # 25 steps per chunk
CCH = TCH * BB        # 800 cols per chunk

_built = None


def _build():
    import concourse.bass as bass
    import concourse.tile as tile
    from concourse import bacc, mybir

    f32 = mybir.dt.float32
    bf16 = mybir.dt.bfloat16
    AF = mybir.ActivationFunctionType
    ALU = mybir.AluOpType
    nc = bacc.Bacc("TRN2", target_bir_lowering=False, debug=False, num_devices=NC)
    # ---- DRAM tensors (per-core data fed via in_maps) ----
    dt_in = {}
    for name in ["x", "xl", "mask", "delta", "xlb", "dltb", "xmb"]:
        dt_in[name] = nc.dram_tensor(name, [F, TB], f32, kind="ExternalInput").ap()
    wg_d = nc.dram_tensor("wg", [128, 80 * 128], bf16, kind="ExternalInput").ap()
    # bf16 declared below after dtype aliases
    wq3f_d = nc.dram_tensor("wq3f", [128, 512], f32, kind="ExternalInput").ap()
    memt_d = nc.dram_tensor("memt", [128, 512], f32, kind="ExternalInput").ap()
    membf_d = nc.dram_tensor("membf", [128, 512], bf16, kind="ExternalInput").ap()
    wfct_d = nc.dram_tensor("wfct", [128, 512], f32, kind="ExternalInput").ap()
    wqz_d = nc.dram_tensor("wqz", [128, 128], f32, kind="ExternalInput").ap()
    wqzp_d = nc.dram_tensor("wqzp", [128, 128], f32, kind="ExternalInput").ap()
    biast_d = nc.dram_tensor("biast", [128, 16], f32, kind="ExternalInput").ap()
    scal_d = nc.dram_tensor("scal", [128, 8], f32, kind="ExternalInput").ap()
    # scal cols: 0 dgz, 1 bgz, 2 dgzp, 3 bgzp, 4 b_q_eff, 5 b_fc
    o_d = nc.dram_tensor("o", [O, BB], f32, kind="ExternalOutput").ap()
    dbg = os.environ.get("LG_DEBUG") == "1"
    if dbg:
        dbg_d = {nm: nc.dram_tensor(f"dbg_{nm}", shp, f32, kind="ExternalOutput").ap()
                 for nm, shp in [("lsf", [128, BB]), ("eT", [128, 128]),
                                 ("ssb", [128, BB]), ("gdn", [128, BB]),
                                 ("Y", [128, 512]), ("h", [128, 128]),
                                 ("lsz", [128, BB]), ("z", [128, BB]), ("zp", [128, BB]),
                                 ("G", [128, 512]), ("hbin", [128, 128])]}

    with tile.TileContext(nc) as tc, ExitStack() as ctx:
        wpool = ctx.enter_context(tc.tile_pool(name="wpool", bufs=1))
        inp = ctx.enter_context(tc.tile_pool(name="inp", bufs=2))
        pre = ctx.enter_context(tc.tile_pool(name="pre", bufs=2))
        lszp = ctx.enter_context(tc.tile_pool(name="lszp", bufs=1))
        stp = ctx.enter_context(tc.tile_pool(name="stp", bufs=2))
        state = ctx.enter_context(tc.tile_pool(name="state", bufs=2))
        pers = ctx.enter_context(tc.tile_pool(name="pers", bufs=1))
        attn_ps = ctx.enter_context(tc.tile_pool(name="attn_ps", bufs=2, space="PSUM"))
        gates_ps = ctx.enter_context(tc.tile_pool(name="gates_ps", bufs=2, space="PSUM"))
        pre_ps = ctx.enter_context(tc.tile_pool(name="pre_ps", bufs=2, space="PSUM"))

        # ---- static weights into SBUF ----
        WG = wpool.tile([128, 80 * 128], bf16, tag="WG")
        nc.sync.dma_start(WG[:], wg_d[:])
        WQ3FT = wpool.tile([128, 512], f32, tag="WQ3FT")
        nc.sync.dma_start(WQ3FT[:], wq3f_d[:])
        MEMT = wpool.tile([128, 512], f32, tag="MEMT")
        nc.sync.dma_start(MEMT[:], memt_d[:])
        MEMBF = wpool.tile([128, 512], bf16, tag="MEMBF")
        nc.sync.dma_start(MEMBF[:], membf_d[:])
        WFCT = wpool.tile([128, 512], f32, tag="WFCT")
        nc.sync.dma_start(WFCT[:], wfct_d[:])
        WQZ = wpool.tile([128, 128], f32, tag="WQZ")
        nc.sync.dma_start(WQZ[:], wqz_d[:])
        WQZP = wpool.tile([128, 128], f32, tag="WQZP")
        nc.sync.dma_start(WQZP[:], wqzp_d[:])
        BIAST = wpool.tile([128, 16], f32, tag="BIAST")
        nc.sync.dma_start(BIAST[:], biast_d[:])
        SCAL = wpool.tile([128, 8], f32, tag="SCAL")
        nc.sync.dma_start(SCAL[:], scal_d[:])
        ONESF = wpool.tile([128, 128], bf16, tag="ONESF")
        nc.vector.memset(ONESF[:], 1.0)
        ONESC = wpool.tile([128, 1], bf16, tag="ONESC")
        nc.vector.memset(ONESC[:], 1.0)

        dgz, bgz = SCAL[:, 0:1], SCAL[:, 1:2]
        dgzp, bgzp = SCAL[:, 2:3], SCAL[:, 3:4]
        bq_ap, bfc_ap = SCAL[:, 4:5], SCAL[:, 5:6]

        # ---- persistent tiles ----
        ls_z = lszp.tile([128, TB], f32, tag="ls_z")
        Xpad = pers.tile([128, BB], bf16, tag="Xpad")
        nc.vector.memset(Xpad[:], 0.0)

        h_f = pers.tile([128, 128], f32, tag="h_f")
        h_b = pers.tile([128, 128], bf16, tag="h_b")
        c_t = pers.tile([128, 128], f32, tag="c_t")
        nc.vector.memset(h_f[:], 0.0)
        nc.vector.memset(h_b[:], 0.0)
        nc.vector.memset(c_t[:], 0.0)

        # ---- precompute z/zp and ls_z in T-chunks ----
        with nc.named_scope("precompute"):
            for cc in range(NTCH):
                sl = slice(cc * CCH, (cc + 1) * CCH)
                ch = {}
                for name in ["x", "xl", "mask", "delta", "xlb", "dltb", "xmb"]:
                    t_ = inp.tile([128, CCH], f32, tag=f"in_{name}")
                    nc.sync.dma_start(t_[:], dt_in[name][:, sl])
                    ch[name] = t_

                def zchain(dsrc, xlsrc, dg, bg, tag):
                    r1 = pre.tile([128, CCH], f32, tag="tA")
                    nc.scalar.activation(r1[:], dsrc[:], AF.Relu, scale=dg, bias=bg)
                    dz = pre.tile([128, CCH], f32, tag="tB")
                    nc.scalar.activation(dz[:], r1[:], AF.Exp, scale=-1.0)
                    u = pre.tile([128, CCH], f32, tag="tA")
                    nc.vector.tensor_tensor(u[:], xlsrc[:], ch["xmb"][:], ALU.subtract)
                    v = pre.tile([128, CCH], f32, tag="tB2")
                    nc.vector.tensor_tensor(v[:], dz[:], u[:], ALU.mult)
                    w = pre.tile([128, CCH], f32, tag="tC")
                    nc.vector.tensor_tensor(w[:], v[:], ch["xmb"][:], ALU.add)
                    d_ = pre.tile([128, CCH], f32, tag="tA")
                    nc.vector.tensor_tensor(d_[:], ch["x"][:], w[:], ALU.subtract)
                    e2 = pre.tile([128, CCH], f32, tag="tB")
                    nc.vector.tensor_tensor(e2[:], ch["mask"][:], d_[:], ALU.mult)
                    z_ = pre.tile([128, CCH], f32, tag=f"z{tag}")
                    nc.vector.tensor_tensor(z_[:], w[:], e2[:], ALU.add)
                    return z_

                z_c = zchain(ch["delta"], ch["xl"], dgz, bgz, "z")
                zp_c = zchain(ch["dltb"], ch["xlb"], dgzp, bgzp, "p")
                if dbg and cc == 0:
                    nc.sync.dma_start(dbg_d["z"][:], z_c[:, 0:BB])
                    nc.sync.dma_start(dbg_d["zp"][:], zp_c[:, 0:BB])

                for off in range(0, CCH, 512):
                    n = min(512, CCH - off)
                    pp = pre_ps.tile([128, 512], f32, tag="pp")
                    nc.tensor.matmul(pp[:, :n], lhsT=WQZ[:], rhs=z_c[:, off:off + n],
                                     start=True, stop=False)
                    nc.tensor.matmul(pp[:, :n], lhsT=WQZP[:], rhs=zp_c[:, off:off + n],
                                     start=False, stop=True)
                    nc.scalar.activation(ls_z[:, cc * CCH + off: cc * CCH + off + n],
                                         pp[:, :n], AF.Identity, bias=bq_ap)

        # ---- recurrence ----
        for t in range(T):
            with nc.named_scope(f"step{t}" if t % 10 == 0 else "step"):
                pa = attn_ps.tile([128, 512], f32, tag="pa")
                # ls = ls_z[t] + WQ3F.T @ h   (fp32)
                for k in range(4):
                    nc.tensor.matmul(pa[:, 0:32], lhsT=WQ3FT[:, 128 * k:128 * (k + 1)],
                                     rhs=h_f[:, 32 * k:32 * k + 32],
                                     start=(k == 0), stop=(k == 3))
                lsf = stp.tile([128, BB], f32, tag="lsf")
                nc.vector.tensor_tensor(lsf[:], pa[:, 0:32], ls_z[:, 32 * t:32 * t + 32], ALU.add)
                # logits^T = memory @ ls  (fp32), 4 M-chunks
                for j in range(4):
                    nc.tensor.matmul(pa[:, 128 + 32 * j:128 + 32 * (j + 1)],
                                     lhsT=MEMT[:, 128 * j:128 * (j + 1)], rhs=lsf[:],
                                     start=True, stop=True)
                eT = stp.tile([128, 128], bf16, tag="eT")
                nc.scalar.activation(eT[:], pa[:, 128:256], AF.Exp)
                # sums over M (partition dim) via ones matmul -> [1, 128]
                nc.tensor.matmul(pa[0:1, 320:448], lhsT=ONESC[:], rhs=eT[:],
                                 start=True, stop=True)
                sums = stp.tile([1, BB], f32, tag="sums")
                nc.vector.tensor_reduce(sums[:], pa[0:1, 320:448].rearrange("p (c b) -> p b c", c=4),
                                        axis=mybir.AxisListType.X, op=ALU.add)
                recipf = stp.tile([1, BB], f32, tag="recipf")
                nc.vector.reciprocal(recipf[:], sums[:])
                nc.vector.tensor_copy(Xpad[0:1, :], recipf[:])
                # gd^T = memory.T-chunks @ e^T  (bf16)
                for j in range(4):
                    nc.tensor.matmul(pa[:, 256:288], lhsT=MEMBF[:, 128 * j:128 * (j + 1)],
                                     rhs=eT[:, 32 * j:32 * j + 32],
                                     start=(j == 0), stop=(j == 3))
                # broadcast recip over partitions: ones[128,128].T @ Xpad
                nc.tensor.matmul(pa[:, 288:320], lhsT=ONESF[:], rhs=Xpad[:],
                                 start=True, stop=True)
                s_sb = stp.tile([128, BB], f32, tag="s_sb")
                nc.scalar.activation(s_sb[:], pa[:, 288:320], AF.Identity)
                gdn = stp.tile([128, BB], bf16, tag="gdn")
                nc.vector.tensor_tensor(gdn[:], pa[:, 256:288], s_sb[:], ALU.mult)
                # gates: per-chunk contiguous groups [ih, hh x4]
                pg = gates_ps.tile([128, 512], f32, tag="pg")
                for g in range(16):
                    nc.tensor.matmul(pg[:, 32 * g:32 * g + 32],
                                     lhsT=WG[:, 128 * (g * 5):128 * (g * 5 + 1)],
                                     rhs=gdn[:], start=True, stop=False)
                    for k in range(4):
                        nc.tensor.matmul(pg[:, 32 * g:32 * g + 32],
                                         lhsT=WG[:, 128 * (g * 5 + 1 + k):128 * (g * 5 + 2 + k)],
                                         rhs=h_b[:, 32 * k:32 * k + 32],
                                         start=False, stop=(k == 3))
                # pointwise: Y = tanh(scale*gates + bias')
                Y = stp.tile([128, 512], f32, tag="Y")
                for g in range(16):
                    sc = 1.0 if g // 4 == 2 else 0.5
                    nc.scalar.activation(Y[:, 32 * g:32 * g + 32], pg[:, 32 * g:32 * g + 32],
                                         AF.Tanh, scale=sc, bias=BIAST[:, g:g + 1])
                SI = stp.tile([128, 128], f32, tag="SI")
                nc.vector.tensor_scalar(SI[:], Y[:, 0:128], 1.0, 0.5, ALU.add, ALU.mult)
                SF = stp.tile([128, 128], f32, tag="SF")
                nc.vector.tensor_scalar(SF[:], Y[:, 128:256], 1.0, 0.5, ALU.add, ALU.mult)
                SO = stp.tile([128, 128], f32, tag="SO")
                nc.vector.tensor_scalar(SO[:], Y[:, 384:512], 1.0, 0.5, ALU.add, ALU.mult)
                m1 = stp.tile([128, 128], f32, tag="m1")
                nc.vector.tensor_tensor(m1[:], SF[:], c_t[:], ALU.mult)
                m2 = stp.tile([128, 128], f32, tag="m2")
                nc.vector.tensor_tensor(m2[:], SI[:], Y[:, 256:384], ALU.mult)
                c_new = state.tile([128, 128], f32, tag="c_t2")
                nc.vector.tensor_tensor(c_new[:], m1[:], m2[:], ALU.add)
                TC = stp.tile([128, 128], f32, tag="TC")
                nc.scalar.activation(TC[:], c_new[:], AF.Tanh)
                h_new = state.tile([128, 128], f32, tag="h_f2")
                nc.vector.tensor_tensor(h_new[:], SO[:], TC[:], ALU.mult)
                hb_new = state.tile([128, 128], bf16, tag="h_b2")
                nc.vector.tensor_copy(hb_new[:], h_new[:])
                if dbg and t == int(os.environ.get('LG_DBGT', '0')):
                    Gd = stp.tile([128, 512], f32, tag="Gd")
                    nc.scalar.activation(Gd[:], pg[:], AF.Identity)
                    nc.sync.dma_start(dbg_d["G"][:], Gd[:])
                    hbf = stp.tile([128, 128], f32, tag="hbf")
                    nc.vector.tensor_copy(hbf[:], h_b[:])
                    nc.sync.dma_start(dbg_d["hbin"][:], hbf[:])
                    nc.sync.dma_start(dbg_d["lsz"][:], ls_z[:, 0:BB])
                    nc.sync.dma_start(dbg_d["lsf"][:], lsf[:])
                    eTf = stp.tile([128, 128], f32, tag="eTf")
                    nc.vector.tensor_copy(eTf[:], eT[:])
                    nc.sync.dma_start(dbg_d["eT"][:], eTf[:])
                    nc.sync.dma_start(dbg_d["ssb"][:], s_sb[:])
                    gdnf = stp.tile([128, BB], f32, tag="gdnf")
                    nc.vector.tensor_copy(gdnf[:], gdn[:])
                    nc.sync.dma_start(dbg_d["gdn"][:], gdnf[:])
                    nc.sync.dma_start(dbg_d["Y"][:], Y[:])
                    nc.sync.dma_start(dbg_d["h"][:], h_new[:])
                h_f, h_b, c_t = h_new, hb_new, c_new

        # ---- final output: out^T = W_fc @ h + b_fc ----
        with nc.named_scope("final"):
            pf = attn_ps.tile([128, 512], f32, tag="pa")
            for k in range(4):
                nc.tensor.matmul(pf[:, 0:32], lhsT=WFCT[:, 128 * k:128 * (k + 1)],
                                 rhs=h_f[:, 32 * k:32 * k + 32],
                                 start=(k == 0), stop=(k == 3))
            outt = stp.tile([O, BB], f32, tag="outt")
            nc.scalar.activation(outt[:], pf[:, 0:32], AF.Identity, bias=bfc_ap)
            nc.sync.dma_start(o_d[:], outt[:])

    nc.compile()
    return nc


def _prep_host(inputs):
    """Host-side: fold weights, build per-core input maps."""
    inp = {k: np.asarray(v, np.float32) for k, v in inputs.items()}
    dgz = np.ascontiguousarray(np.diag(inp["W_gz"]))
    dgzp = np.ascontiguousarray(np.diag(inp["W_gzp"]))
    Wq = inp["W_q"]
    WQ3F = (Wq[:, 2 * F:] @ inp["W_fc"]).astype(np.float32)       # [F, H]
    b_q_eff = (inp["b_q"] + Wq[:, 2 * F:] @ inp["b_fc"]).astype(np.float32)
    bias_g = (inp["b_ih"] + inp["b_hh"]).astype(np.float32)       # [2048]

    # gates weights: Wcat.T tiles; tile (g, k): k=0 -> W_ih cols, k=1..4 -> W_hh
    WcatT = np.concatenate([inp["W_ih"], inp["W_hh"]], axis=1).T  # [640, 2048]
    wg = np.empty((128, 80 * 128), np.float32)
    for g in range(16):
        for k in range(5):
            wg[:, 128 * (g * 5 + k):128 * (g * 5 + k + 1)] = \
                WcatT[128 * k:128 * (k + 1), 128 * g:128 * (g + 1)]

    wq3f = np.empty((128, 512), np.float32)    # (WQ3F.T) chunks [128hk, F]
    for k in range(4):
        wq3f[:, 128 * k:128 * (k + 1)] = WQ3F.T[128 * k:128 * (k + 1), :]
    memt = np.ascontiguousarray(inp["memory"].T)                  # [F, M] = [128, 512]
    membf = np.empty((128, 512), np.float32)   # memory row-chunks [m_local, F]
    for j in range(4):
        membf[:, 128 * j:128 * (j + 1)] = inp["memory"][128 * j:128 * (j + 1), :]
    wfct = np.empty((128, 512), np.float32)    # W_fc.T chunks [128hk, O]
    for k in range(4):
        wfct[:, 128 * k:128 * (k + 1)] = inp["W_fc"].T[128 * k:128 * (k + 1), :]
    wqz = np.ascontiguousarray(Wq[:, 0:128].T)
    wqzp = np.ascontiguousarray(Wq[:, 128:256].T)

    biast = np.empty((128, 16), np.float32)
    for g in range(16):
        sc = 1.0 if g // 4 == 2 else 0.5
        biast[:, g] = bias_g[128 * g:128 * (g + 1)] * sc

    scal = np.zeros((128, 8), np.float32)
    scal[:, 0], scal[:, 1] = dgz, inp["b_gz"]
    scal[:, 2], scal[:, 3] = dgzp, inp["b_gzp"]
    scal[:, 4], scal[:, 5] = b_q_eff, inp["b_fc"]

    import ml_dtypes
    wg = wg.astype(ml_dtypes.bfloat16)
    membf = membf.astype(ml_dtypes.bfloat16)
    shared = dict(wg=wg, wq3f=wq3f, memt=memt, membf=membf, wfct=wfct,
                  wqz=wqz, wqzp=wqzp, biast=biast, scal=scal)

    xm_rep = np.ascontiguousarray(
        np.repeat(inp["X_mean"][:T].T[:, :, None], BB, axis=2).reshape(F, TB))
    in_maps = []
    ch_names = ["x", "xl", "mask", "delta", "xlb", "dltb"]
    ch_idx = [0, 1, 2, 3, 4, 5]
    for core in range(NC):
        b0 = core * BB
        m = dict(shared)
        sl = inp["input"][b0:b0 + BB]          # [BB, 6, 100, F]
        for nm, ci in zip(ch_names, ch_idx):
            # [F, T, BB] -> [F, T*BB]
            m[nm] = np.ascontiguousarray(
                np.transpose(sl[:, ci, :T], (2, 1, 0)).reshape(F, TB))
        m["xmb"] = xm_rep
        in_maps.append(m)
    return in_maps


def kernel(**inputs):
    global _built
    from concourse import bass_utils
    if _built is None:
        _built = _build()
    in_maps = _prep_host(inputs)
    res = bass_utils.run_bass_kernel_spmd(_built, in_maps, core_ids=list(range(NC)))
    out = np.empty((B, 1, O), np.float32)
    for core in range(NC):
        out[core * BB:(core + 1) * BB, 0, :] = res.results[core]["o"].T
    return out



# revision 7
# speedup vs baseline: 1.7004x; 1.7004x over previous
"""Trainium2 Bass kernel for nn_LGnet (LSTM + memory attention recurrence).

Sharding: data-parallel over batch, B=256 -> 32 rows per core across 8 cores.
All on-chip state is kept transposed ([feature partitions, batch free]).

Step structure (per t):
  ls   = ls_z[t] + WQ3F.T @ h          (4 bf16 matmuls; WQ3F pre-scaled 0.5)
  logits = memory @ ls                  (4 fp32 matmuls - precision critical)
  e = exp(logits); s = ones-matmul sum; r = 1/s (bcast via ones matmul)
  gd = (e @ memory) * r                 (bf16 matmuls)
  gates = bias + W_hh' @ h2 + W_ih' @ gdn   (96 bf16 matmuls, bias via
            rank-1 ones matmuls; i/f/o rows pre-scaled 0.5, W_hh cols 0.5)
  Y = tanh(gates)                       (ONE [128,512] activation)
  u = (Yf+1)*ch; v = (Yi+1)*Yg; cn2 = 2u+v          (fused scalar_tensor_tensor)
  TC = tanh(0.5*cn2); hb = (Yo+1)*TC -> bf16 (=2h); ch = 0.25*cn2 (=c/2)
The h-dependent gate matmuls are interleaved into the softmax window on the
tensor engine so they never sit on the critical path.
"""
import os
import numpy as np
from contextlib import ExitStack

B, T, F, H, O, M = 256, 100, 128, 512, 128, 512
T = int(os.environ.get("LG_T", str(T)))   # debug override; harness uses 100
NC = 8
BB = B // NC          # 32 batch rows per core
TB = T * BB           # 3200 columns in (t, b) packing
NTCH = 4              # precompute T-chunks
TCH = T // NTCH       # 25 steps per chunk
CCH = TCH * BB        # 800 cols per chunk

_built = None


def _build():
    import concourse.bass as bass
    import concourse.tile as tile
    from concourse import bacc, mybir

    f32 = mybir.dt.float32
    bf16 = mybir.dt.bfloat16
    AF = mybir.ActivationFunctionType
    ALU = mybir.AluOpType
    nc = bacc.Bacc("TRN2", target_bir_lowering=False, debug=False, num_devices=NC)
    # ---- DRAM tensors (per-core data fed via in_maps) ----
    dt_in = {}
    for name in ["x", "xl", "mask", "delta", "xlb", "dltb", "xmb"]:
        dt_in[name] = nc.dram_tensor(name, [F, TB], f32, kind="ExternalInput").ap()
    wg_d = nc.dram_tensor("wg", [128, 80 * 128], bf16, kind="ExternalInput").ap()
    selm_d = nc.dram_tensor("selm", [16, 512], bf16, kind="ExternalInput").ap()
    biast16_d = nc.dram_tensor("biast16", [16, 128], bf16, kind="ExternalInput").ap()
    wq3f_d = nc.dram_tensor("wq3f", [128, 512], bf16, kind="ExternalInput").ap()
    memt_d = nc.dram_tensor("memt", [128, 512], f32, kind="ExternalInput").ap()
    membf_d = nc.dram_tensor("membf", [128, 512], bf16, kind="ExternalInput").ap()
    wfct_d = nc.dram_tensor("wfct", [128, 512], f32, kind="ExternalInput").ap()
    wqz_d = nc.dram_tensor("wqz", [128, 128], f32, kind="ExternalInput").ap()
    wqzp_d = nc.dram_tensor("wqzp", [128, 128], f32, kind="ExternalInput").ap()
    scal_d = nc.dram_tensor("scal", [128, 8], f32, kind="ExternalInput").ap()
    # scal cols: 0 dgz, 1 bgz, 2 dgzp, 3 bgzp, 4 b_q_eff, 5 b_fc
    o_d = nc.dram_tensor("o", [O, BB], f32, kind="ExternalOutput").ap()
    dbg = os.environ.get("LG_DEBUG") == "1"
    if dbg:
        dbg_d = {nm: nc.dram_tensor(f"dbg_{nm}", shp, f32, kind="ExternalOutput").ap()
                 for nm, shp in [("lsf", [128, BB]), ("eT", [128, 128]),
                                 ("gdn", [128, BB]), ("G", [128, 512]),
                                 ("Y", [128, 512]), ("cn2", [128, 128]),
                                 ("hb", [128, 128])]}

    with tile.TileContext(nc) as tc, ExitStack() as ctx:
        wpool = ctx.enter_context(tc.tile_pool(name="wpool", bufs=1))
        inp = ctx.enter_context(tc.tile_pool(name="inp", bufs=2))
        pre = ctx.enter_context(tc.tile_pool(name="pre", bufs=2))
        lszp = ctx.enter_context(tc.tile_pool(name="lszp", bufs=1))
        stp = ctx.enter_context(tc.tile_pool(name="stp", bufs=2))
        state = ctx.enter_context(tc.tile_pool(name="state", bufs=2))
        pers = ctx.enter_context(tc.tile_pool(name="pers", bufs=1))
        attn_ps = ctx.enter_context(tc.tile_pool(name="attn_ps", bufs=2, space="PSUM"))
        gates_ps = ctx.enter_context(tc.tile_pool(name="gates_ps", bufs=2, space="PSUM"))
        pre_ps = ctx.enter_context(tc.tile_pool(name="pre_ps", bufs=2, space="PSUM"))

        # ---- static weights into SBUF ----
        WG = wpool.tile([128, 80 * 128], bf16, tag="WG")
        nc.sync.dma_start(WG[:], wg_d[:])
        SELM = wpool.tile([16, 512], bf16, tag="SELM")
        nc.sync.dma_start(SELM[:], selm_d[:])
        BIAST16 = wpool.tile([16, 128], bf16, tag="BIAST16")
        nc.sync.dma_start(BIAST16[:], biast16_d[:])
        WQ3FT = wpool.tile([128, 512], bf16, tag="WQ3FT")
        nc.sync.dma_start(WQ3FT[:], wq3f_d[:])
        MEMT = wpool.tile([128, 512], f32, tag="MEMT")
        nc.sync.dma_start(MEMT[:], memt_d[:])
        MEMBF = wpool.tile([128, 512], bf16, tag="MEMBF")
        nc.sync.dma_start(MEMBF[:], membf_d[:])
        WFCT = wpool.tile([128, 512], f32, tag="WFCT")
        nc.sync.dma_start(WFCT[:], wfct_d[:])
        WQZ = wpool.tile([128, 128], f32, tag="WQZ")
        nc.sync.dma_start(WQZ[:], wqz_d[:])
        WQZP = wpool.tile([128, 128], f32, tag="WQZP")
        nc.sync.dma_start(WQZP[:], wqzp_d[:])
        SCAL = wpool.tile([128, 8], f32, tag="SCAL")
        nc.sync.dma_start(SCAL[:], scal_d[:])
        ONESF = wpool.tile([128, 128], bf16, tag="ONESF")
        nc.vector.memset(ONESF[:], 1.0)
        ONESC = wpool.tile([128, 1], bf16, tag="ONESC")
        nc.vector.memset(ONESC[:], 1.0)

        dgz, bgz = SCAL[:, 0:1], SCAL[:, 1:2]
        dgzp, bgzp = SCAL[:, 2:3], SCAL[:, 3:4]
        bq_ap, bfc_ap = SCAL[:, 4:5], SCAL[:, 5:6]

        # ---- persistent tiles ----
        ls_z = lszp.tile([128, TB], f32, tag="ls_z")
        Xpad = pers.tile([128, BB], bf16, tag="Xpad")
        nc.vector.memset(Xpad[:], 0.0)

        hb = pers.tile([128, 128], bf16, tag="hb")     # 2h, bf16
        ch = pers.tile([128, 128], f32, tag="ch")      # c/2, fp32
        nc.vector.memset(hb[:], 0.0)
        nc.vector.memset(ch[:], 0.0)

        # ---- precompute z/zp and ls_z in T-chunks ----
        with nc.named_scope("precompute"):
            for cc in range(NTCH):
                sl = slice(cc * CCH, (cc + 1) * CCH)
                chd = {}
                for name in ["x", "xl", "mask", "delta", "xlb", "dltb", "xmb"]:
                    t_ = inp.tile([128, CCH], f32, tag=f"in_{name}")
                    nc.sync.dma_start(t_[:], dt_in[name][:, sl])
                    chd[name] = t_

                def zchain(dsrc, xlsrc, dg, bg, tag):
                    r1 = pre.tile([128, CCH], f32, tag="tA")
                    nc.scalar.activation(r1[:], dsrc[:], AF.Relu, scale=dg, bias=bg)
                    dz = pre.tile([128, CCH], f32, tag="tB")
                    nc.scalar.activation(dz[:], r1[:], AF.Exp, scale=-1.0)
                    u = pre.tile([128, CCH], f32, tag="tA")
                    nc.vector.tensor_tensor(u[:], xlsrc[:], chd["xmb"][:], ALU.subtract)
                    v = pre.tile([128, CCH], f32, tag="tB2")
                    nc.vector.tensor_tensor(v[:], dz[:], u[:], ALU.mult)
                    w = pre.tile([128, CCH], f32, tag="tC")
                    nc.vector.tensor_tensor(w[:], v[:], chd["xmb"][:], ALU.add)
                    d_ = pre.tile([128, CCH], f32, tag="tA")
                    nc.vector.tensor_tensor(d_[:], chd["x"][:], w[:], ALU.subtract)
                    e2 = pre.tile([128, CCH], f32, tag="tB")
                    nc.vector.tensor_tensor(e2[:], chd["mask"][:], d_[:], ALU.mult)
                    z_ = pre.tile([128, CCH], f32, tag=f"z{tag}")
                    nc.vector.tensor_tensor(z_[:], w[:], e2[:], ALU.add)
                    return z_

                z_c = zchain(chd["delta"], chd["xl"], dgz, bgz, "z")
                zp_c = zchain(chd["dltb"], chd["xlb"], dgzp, bgzp, "p")

                for off in range(0, CCH, 512):
                    n = min(512, CCH - off)
                    pp = pre_ps.tile([128, 512], f32, tag="pp")
                    nc.tensor.matmul(pp[:, :n], lhsT=WQZ[:], rhs=z_c[:, off:off + n],
                                     start=True, stop=False)
                    nc.tensor.matmul(pp[:, :n], lhsT=WQZP[:], rhs=zp_c[:, off:off + n],
                                     start=False, stop=True)
                    nc.scalar.activation(ls_z[:, cc * CCH + off: cc * CCH + off + n],
                                         pp[:, :n], AF.Identity, bias=bq_ap)

        # gate-group matmul helpers ------------------------------------
        def g_h(pg, g):
            # 4 h-chunk matmuls accumulating (RMW) onto the bias written by
            # the single selector matmul
            for k in range(4):
                nc.tensor.matmul(pg[:, 32 * g:32 * g + 32],
                                 lhsT=WG[:, 128 * (g * 5 + 1 + k):128 * (g * 5 + 2 + k)],
                                 rhs=hb[:, 32 * k:32 * k + 32],
                                 start=False, stop=False, skip_group_check=True)

        def g_gd(pg, g, gdn):
            nc.tensor.matmul(pg[:, 32 * g:32 * g + 32],
                             lhsT=WG[:, 128 * (g * 5):128 * (g * 5 + 1)],
                             rhs=gdn[:], start=False, stop=True, skip_group_check=True)

        # ---- recurrence ----
        for t in range(T):
            with nc.named_scope(f"step{t}" if t % 10 == 0 else "step"):
                pa = attn_ps.tile([128, 512], f32, tag="pa")
                pg = gates_ps.tile([128, 512], f32, tag="pg")
                # ls = ls_z[t] + WQ3F.T @ h   (bf16)
                for k in range(4):
                    nc.tensor.matmul(pa[:, 0:32], lhsT=WQ3FT[:, 128 * k:128 * (k + 1)],
                                     rhs=hb[:, 32 * k:32 * k + 32],
                                     start=(k == 0), stop=(k == 3))
                lsf = stp.tile([128, BB], f32, tag="lsf")
                nc.vector.tensor_tensor(lsf[:], pa[:, 0:32], ls_z[:, 32 * t:32 * t + 32], ALU.add)
                # logits^T = memory @ ls  (fp32), 4 M-chunks
                for j in range(4):
                    nc.tensor.matmul(pa[:, 128 + 32 * j:128 + 32 * (j + 1)],
                                     lhsT=MEMT[:, 128 * j:128 * (j + 1)], rhs=lsf[:],
                                     start=True, stop=True)
                eT = stp.tile([128, 128], bf16, tag="eT")
                nc.scalar.activation(eT[:], pa[:, 128:256], AF.Exp)
                # bias for all 16 groups via one K=16 selector matmul (single
                # closed accumulation group; everything after is RMW)
                nc.tensor.matmul(pg[:, 0:512], lhsT=BIAST16[:], rhs=SELM[:],
                                 start=True, stop=True)
                # gates: h part for groups 0..3 (overlaps exp)
                for g in range(0, 4):
                    g_h(pg, g)
                # sums over M (partition dim) via ones matmul -> [1, 128]
                nc.tensor.matmul(pa[0:1, 320:448], lhsT=ONESC[:], rhs=eT[:],
                                 start=True, stop=True)
                sums = stp.tile([1, BB], f32, tag="sums")
                nc.vector.tensor_reduce(sums[:], pa[0:1, 320:448].rearrange("p (c b) -> p b c", c=4),
                                        axis=mybir.AxisListType.X, op=ALU.add)
                recipf = stp.tile([1, BB], f32, tag="recipf")
                nc.vector.reciprocal(recipf[:], sums[:])
                nc.vector.tensor_copy(Xpad[0:1, :], recipf[:])
                # gates groups 4..9
                for g in range(4, 10):
                    g_h(pg, g)
                # gd^T = memory.T-chunks @ e^T  (bf16)
                for j in range(4):
                    nc.tensor.matmul(pa[:, 256:288], lhsT=MEMBF[:, 128 * j:128 * (j + 1)],
                                     rhs=eT[:, 32 * j:32 * j + 32],
                                     start=(j == 0), stop=(j == 3))
                # broadcast recip over partitions: ones[128,128].T @ Xpad
                nc.tensor.matmul(pa[:, 288:320], lhsT=ONESF[:], rhs=Xpad[:],
                                 start=True, stop=True)
                s_sb = stp.tile([128, BB], f32, tag="s_sb")
                nc.scalar.activation(s_sb[:], pa[:, 288:320], AF.Identity)
                gdn = stp.tile([128, BB], bf16, tag="gdn")
                nc.vector.tensor_tensor(gdn[:], pa[:, 256:288], s_sb[:], ALU.mult)
                # gates groups 10..15, then gd part for all groups
                for g in range(10, 16):
                    g_h(pg, g)
                for g in range(16):
                    g_gd(pg, g, gdn)
                # ONE tanh over all gates (weights/bias pre-scaled)
                Y = stp.tile([128, 512], f32, tag="Y")
                nc.scalar.activation(Y[:], pg[:], AF.Tanh)
                # pointwise: u=(Yf+1)*ch  v=(Yi+1)*Yg  cn2=2u+v
                u = stp.tile([128, 128], f32, tag="u")
                nc.vector.scalar_tensor_tensor(u[:], Y[:, 128:256], 1.0, ch[:],
                                               ALU.add, ALU.mult)
                v = stp.tile([128, 128], f32, tag="v")
                nc.vector.scalar_tensor_tensor(v[:], Y[:, 0:128], 1.0, Y[:, 256:384],
                                               ALU.add, ALU.mult)
                cn2 = state.tile([128, 128], f32, tag="cn2")
                nc.vector.scalar_tensor_tensor(cn2[:], u[:], 2.0, v[:],
                                               ALU.mult, ALU.add)
                TC = stp.tile([128, 128], f32, tag="TC")
                nc.scalar.activation(TC[:], cn2[:], AF.Tanh, scale=0.5)
                hb_new = state.tile([128, 128], bf16, tag="hb2")
                nc.vector.scalar_tensor_tensor(hb_new[:], Y[:, 384:512], 1.0, TC[:],
                                               ALU.add, ALU.mult)
                ch_new = state.tile([128, 128], f32, tag="ch2")
                nc.scalar.mul(ch_new[:], cn2[:], 0.25)
                if dbg and t == int(os.environ.get("LG_DBGT", "0")):
                    nc.sync.dma_start(dbg_d["lsf"][:], lsf[:])
                    eTf = stp.tile([128, 128], f32, tag="eTf")
                    nc.vector.tensor_copy(eTf[:], eT[:])
                    nc.sync.dma_start(dbg_d["eT"][:], eTf[:])
                    gdnf = stp.tile([128, BB], f32, tag="gdnf")
                    nc.vector.tensor_copy(gdnf[:], gdn[:])
                    nc.sync.dma_start(dbg_d["gdn"][:], gdnf[:])
                    Gd = stp.tile([128, 512], f32, tag="Gd")
                    nc.scalar.activation(Gd[:], pg[:], AF.Identity)
                    nc.sync.dma_start(dbg_d["G"][:], Gd[:])
                    nc.sync.dma_start(dbg_d["Y"][:], Y[:])
                    nc.sync.dma_start(dbg_d["cn2"][:], cn2[:])
                    hbf = stp.tile([128, 128], f32, tag="hbf")
                    nc.vector.tensor_copy(hbf[:], hb_new[:])
                    nc.sync.dma_start(dbg_d["hb"][:], hbf[:])
                if t == T - 1:
                    h2f = stp.tile([128, 128], f32, tag="h2f")
                    nc.vector.scalar_tensor_tensor(h2f[:], Y[:, 384:512], 1.0, TC[:],
                                                   ALU.add, ALU.mult)
                hb, ch = hb_new, ch_new

        # ---- final output: out^T = 0.5*W_fc @ h2 + b_fc ----
        with nc.named_scope("final"):
            pf = attn_ps.tile([128, 512], f32, tag="pa")
            for k in range(4):
                nc.tensor.matmul(pf[:, 0:32], lhsT=WFCT[:, 128 * k:128 * (k + 1)],
                                 rhs=h2f[:, 32 * k:32 * k + 32],
                                 start=(k == 0), stop=(k == 3))
            outt = stp.tile([O, BB], f32, tag="outt")
            nc.scalar.activation(outt[:], pf[:, 0:32], AF.Identity, bias=bfc_ap)
            nc.sync.dma_start(o_d[:], outt[:])

    nc.compile()
    return nc


def _prep_host(inputs):
    """Host-side: fold weights, build per-core input maps."""
    inp = {k: np.asarray(v, np.float32) for k, v in inputs.items()}
    dgz = np.ascontiguousarray(np.diag(inp["W_gz"]))
    dgzp = np.ascontiguousarray(np.diag(inp["W_gzp"]))
    Wq = inp["W_q"]
    WQ3F = (Wq[:, 2 * F:] @ inp["W_fc"]).astype(np.float32)       # [F, H]
    b_q_eff = (inp["b_q"] + Wq[:, 2 * F:] @ inp["b_fc"]).astype(np.float32)
    bias_g = (inp["b_ih"] + inp["b_hh"]).astype(np.float32)       # [2048]

    # gates weights: Wcat.T tiles; tile (g, k): k=0 -> W_ih cols, k=1..4 -> W_hh
    # pre-scale: i/f/o gate columns (g//4 != 2) x0.5 for the tanh-sigmoid trick,
    # W_hh part (k>=1) x0.5 because h state is stored as 2h.
    WcatT = np.concatenate([inp["W_ih"], inp["W_hh"]], axis=1).T  # [640, 2048]
    wg = np.empty((128, 80 * 128), np.float32)
    for g in range(16):
        gate_sc = 0.5 if g // 4 != 2 else 1.0
        for k in range(5):
            sc = gate_sc * (0.5 if k >= 1 else 1.0)
            wg[:, 128 * (g * 5 + k):128 * (g * 5 + k + 1)] = \
                sc * WcatT[128 * k:128 * (k + 1), 128 * g:128 * (g + 1)]

    selm = np.zeros((16, 512), np.float32)
    for g in range(16):
        selm[g, 32 * g:32 * (g + 1)] = 1.0
    biast16 = np.empty((16, 128), np.float32)
    for g in range(16):
        gate_sc = 0.5 if g // 4 != 2 else 1.0
        biast16[g, :] = gate_sc * bias_g[128 * g:128 * (g + 1)]

    wq3f = np.empty((128, 512), np.float32)    # (0.5*WQ3F.T) chunks [128hk, F]
    for k in range(4):
        wq3f[:, 128 * k:128 * (k + 1)] = 0.5 * WQ3F.T[128 * k:128 * (k + 1), :]
    memt = np.ascontiguousarray(inp["memory"].T)                  # [F, M] = [128, 512]
    membf = np.empty((128, 512), np.float32)   # memory row-chunks [m_local, F]
    for j in range(4):
        membf[:, 128 * j:128 * (j + 1)] = inp["memory"][128 * j:128 * (j + 1), :]
    wfct = np.empty((128, 512), np.float32)    # 0.5*W_fc.T chunks [128hk, O]
    for k in range(4):
        wfct[:, 128 * k:128 * (k + 1)] = 0.5 * inp["W_fc"].T[128 * k:128 * (k + 1), :]
    wqz = np.ascontiguousarray(Wq[:, 0:128].T)
    wqzp = np.ascontiguousarray(Wq[:, 128:256].T)

    scal = np.zeros((128, 8), np.float32)
    scal[:, 0], scal[:, 1] = dgz, inp["b_gz"]
    scal[:, 2], scal[:, 3] = dgzp, inp["b_gzp"]
    scal[:, 4], scal[:, 5] = b_q_eff, inp["b_fc"]

    import ml_dtypes
    wg = wg.astype(ml_dtypes.bfloat16)
    selm = selm.astype(ml_dtypes.bfloat16)
    biast16 = biast16.astype(ml_dtypes.bfloat16)
    wq3f = wq3f.astype(ml_dtypes.bfloat16)
    membf = membf.astype(ml_dtypes.bfloat16)
    shared = dict(wg=wg, selm=selm, biast16=biast16, wq3f=wq3f, memt=memt, membf=membf,
                  wfct=wfct, wqz=wqz, wqzp=wqzp, scal=scal)

    xm_rep = np.ascontiguousarray(
        np.repeat(inp["X_mean"][:T].T[:, :, None], BB, axis=2).reshape(F, TB))
    in_maps = []
    ch_names = ["x", "xl", "mask", "delta", "xlb", "dltb"]
    ch_idx = [0, 1, 2, 3, 4, 5]
    for core in range(NC):
        b0 = core * BB
        m = dict(shared)
        sl = inp["input"][b0:b0 + BB]          # [BB, 6, 100, F]
        for nm, ci in zip(ch_names, ch_idx):
            # [F, T, BB] -> [F, T*BB]
            m[nm] = np.ascontiguousarray(
                np.transpose(sl[:, ci, :T], (2, 1, 0)).reshape(F, TB))
        m["xmb"] = xm_rep
        in_maps.append(m)
    return in_maps


def kernel(**inputs):
    global _built
    from concourse import bass_utils
    if _built is None:
        _built = _build()
    in_maps = _prep_host(inputs)
    res = bass_utils.run_bass_kernel_spmd(_built, in_maps, core_ids=list(range(NC)))
    out = np.empty((B, 1, O), np.float32)
    for core in range(NC):
        out[core * BB:(core + 1) * BB, 0, :] = res.results[core]["o"].T
    return out
